# revision 31
# baseline (speedup 1.0000x reference)
"""DenoiseLSTM Trainium2 kernel (8 NeuronCores, SPMD) — v2.

Strategy vs v1 baseline:
- Input projections are folded into the recurrent loops as extra matmul
  accumulation steps (embeddings live in SBUF from the gather), killing the
  DMA-scatter projection phase and the per-step g=ps+xt vector add.
- Gate m-chunks are permuted host-side to [i, f, o, g] so one sigmoid
  activation covers i/f/o straight out of PSUM; tanh covers g.
- Attention + FFN + vocab projection are restructured per 32-step t-block
  and emitted interleaved with decoder steps, so the Tile scheduler fills
  the decoder's dependency-stall gaps with post-work.
- Attention scores pack 4 batch rows into one [128,S] PSUM tile via
  tile_position column packing; softmax runs on full 128 partitions.
- Output is written bf16 and upcast on host (halves output DMA).
- Optional: recurrent Whh weights in fp8-e4m3 (halves LDWEIGHTS traffic);
  CPU-simulated rel err 0.0079 vs 2e-2 budget.

Replicated recurrences on all 8 cores; vocab projection sharded over V
(4000 columns per core); host concatenates V-shards.
"""
import sys

sys.path.insert(0, "/opt/trn_rl_repo")

from contextlib import ExitStack

import numpy as np
import ml_dtypes

import concourse.bass as bass
import concourse.bacc as bacc
import concourse.mybir as mybir
import concourse.tile as tile
from concourse.bass_utils import run_bass_kernel_spmd
from concourse.masks import make_identity

bf16 = ml_dtypes.bfloat16
f8e4 = ml_dtypes.float8_e4m3
F32 = mybir.dt.float32
BF16 = mybir.dt.bfloat16
FP8E4 = mybir.dt.float8e4
I16 = mybir.dt.int16
AF = mybir.ActivationFunctionType
ALU = mybir.AluOpType
AX = mybir.AxisListType

B = 32
D_EMB = 128
D_ENC = 256
D_DEC = 512
N_CORES = 8
KE = D_ENC // 128   # 2
KD = D_DEC // 128   # 4
ME = 4 * D_ENC // 128  # 8 gate chunks (encoder)
MD = 4 * D_DEC // 128  # 16 gate chunks (decoder)
# m-chunk permutation: [i-chunks, f-chunks, o-chunks, g-chunks]
PERM_E = [0, 1, 2, 3, 6, 7, 4, 5]
PERM_D = [0, 1, 2, 3, 4, 5, 6, 7, 12, 13, 14, 15, 8, 9, 10, 11]


class _Stop(Exception):
    pass


def build(S=128, T=128, V=32000, VS=4000, phases=6, fp8=False, bias_mode=False):
    NI_E = B * S
    NI_D = B * T
    Tc = 32                      # timesteps per post-work block
    NBLK = T // Tc               # 4
    BG = 4                       # batch rows packed per score tile
    SCALE = 0.5 / float(np.sqrt(np.float32(2 * D_ENC)))
    WHH_DT = FP8E4 if fp8 else BF16

    nc = bacc.Bacc("TRN2", target_bir_lowering=False, debug=False)

    # ---- external inputs ----
    tokb = nc.dram_tensor("tokb", [V, D_EMB], BF16, kind="ExternalInput")
    idx_e = nc.dram_tensor("idx_e", [128, NI_E // 16], I16, kind="ExternalInput")
    idx_d = nc.dram_tensor("idx_d", [128, NI_D // 16], I16, kind="ExternalInput")
    startT = nc.dram_tensor("startT", [128, 1], BF16, kind="ExternalInput")
    diff_e = nc.dram_tensor("diff_e", [1, 2 * D_ENC], BF16, kind="ExternalInput")
    e0T = nc.dram_tensor("e0T", [128, KD], F32, kind="ExternalInput")
    lab_i = nc.dram_tensor("lab_i", [1, B], BF16, kind="ExternalInput")
    diff_s = nc.dram_tensor("diff_s", [1, D_DEC], BF16, kind="ExternalInput")
    s0T = nc.dram_tensor("s0T", [128, KD], F32, kind="ExternalInput")
    lab_d = nc.dram_tensor("lab_d", [1, B], BF16, kind="ExternalInput")
    # input-projection weights (bf16), m-perm-major
    wih_f = nc.dram_tensor("wih_f", [128, ME * 128], BF16, kind="ExternalInput")
    wih_b = nc.dram_tensor("wih_b", [128, ME * 128], BF16, kind="ExternalInput")
    wih_d = nc.dram_tensor("wih_d", [128, MD * 128], BF16, kind="ExternalInput")
    # recurrent weights, (m-perm, k)-major, optionally fp8
    whh_f = nc.dram_tensor("whh_f", [128, ME * KE * 128], WHH_DT, kind="ExternalInput")
    whh_b = nc.dram_tensor("whh_b", [128, ME * KE * 128], WHH_DT, kind="ExternalInput")
    whh_d = nc.dram_tensor("whh_d", [128, MD * KD * 128], WHH_DT, kind="ExternalInput")
    wtr = nc.dram_tensor("wtr", [128, KD * KD * 128], BF16, kind="ExternalInput")
    wf1 = nc.dram_tensor("wf1", [128, 8 * KD * 128], BF16, kind="ExternalInput")
    wf2 = nc.dram_tensor("wf2", [128, KD * VS], BF16, kind="ExternalInput")
    b1a = nc.dram_tensor("b1a", [128, KD], F32, kind="ExternalInput")
    b1h = nc.dram_tensor("b1h", [128, KD], F32, kind="ExternalInput")
    if bias_mode:
        brow_f = nc.dram_tensor("brow_f", [1, ME * 128], BF16, kind="ExternalInput")
        brow_b = nc.dram_tensor("brow_b", [1, ME * 128], BF16, kind="ExternalInput")
        brow_d = nc.dram_tensor("brow_d", [1, MD * 128], BF16, kind="ExternalInput")

    out = nc.dram_tensor("out", [B, T, VS], BF16, kind="ExternalOutput")

    with tile.TileContext(nc) as tc, ExitStack() as ctx:
        wpool = ctx.enter_context(tc.tile_pool(name="weights", bufs=1))
        spool = ctx.enter_context(tc.tile_pool(name="state", bufs=1))
        big = ctx.enter_context(tc.tile_pool(name="big", bufs=1))

        def load(dram, shape, dtype, tag):
            t = wpool.tile(shape, dtype, tag=tag, name=tag)
            nc.sync.dma_start(t[:], dram[:, :])
            return t

        # gathers first: idx DMAs + encoder-token gather start immediately,
        # overlapping the weight streams
        idx_e_s = wpool.tile([128, NI_E // 16], I16, tag="idx_e", name="idx_e")
        nc.sync.dma_start(idx_e_s[:], idx_e[:, :])
        idx_d_s = wpool.tile([128, NI_D // 16], I16, tag="idx_d", name="idx_d")
        nc.sync.dma_start(idx_d_s[:], idx_d[:, :])
        encT = big.tile([128, 1, NI_E], BF16, tag="encT", name="encT")
        NQ = NI_E // 4

        def sub_gather(gc):
            nc.gpsimd.dma_gather(encT[:, :, gc * NQ:(gc + 1) * NQ], tokb[:, :],
                                 idx_e_s[:, gc * NQ // 16:(gc + 1) * NQ // 16],
                                 NQ, NQ, D_EMB, transpose=True,
                                 single_packet=False)

        sub_gather(0)
        sub_gather(3)
        decT = big.tile([128, 1, NI_D], BF16, tag="decT", name="decT")

        wih_f_s = load(wih_f, [128, ME * 128], BF16, "wih_f")
        wih_b_s = load(wih_b, [128, ME * 128], BF16, "wih_b")
        wih_d_s = load(wih_d, [128, MD * 128], BF16, "wih_d")
        whh_f_s = load(whh_f, [128, ME * KE * 128], WHH_DT, "whh_f")
        whh_b_s = load(whh_b, [128, ME * KE * 128], WHH_DT, "whh_b")
        whh_d_s = load(whh_d, [128, MD * KD * 128], WHH_DT, "whh_d")
        wtr_s = load(wtr, [128, KD * KD * 128], BF16, "wtr")
        wf1_s = load(wf1, [128, 8 * KD * 128], BF16, "wf1")
        wf2_s = load(wf2, [128, KD * VS], BF16, "wf2")
        b1a_s = load(b1a, [128, KD], F32, "b1a")
        b1h_s = load(b1h, [128, KD], F32, "b1h")
        e0T_s = load(e0T, [128, KD], F32, "e0T")
        s0T_s = load(s0T, [128, KD], F32, "s0T")
        startT_s = load(startT, [128, 1], BF16, "startT")
        ident = wpool.tile([128, 128], BF16, tag="ident", name="ident")
        make_identity(nc, ident)

        diff_e_s = wpool.tile([1, 2 * D_ENC], BF16, tag="diff_e", name="diff_e")
        nc.sync.dma_start(diff_e_s[:], diff_e[:, :])
        diff_s_s = wpool.tile([1, D_DEC], BF16, tag="diff_s", name="diff_s")
        nc.sync.dma_start(diff_s_s[:], diff_s[:, :])
        lab_i_s = wpool.tile([1, B], BF16, tag="lab_i", name="lab_i")
        nc.sync.dma_start(lab_i_s[:], lab_i[:, :])
        lab_d_s = wpool.tile([1, B], BF16, tag="lab_d", name="lab_d")
        nc.sync.dma_start(lab_d_s[:], lab_d[:, :])
        if bias_mode:
            brow_f_s = load(brow_f, [1, ME * 128], BF16, "brow_f")
            brow_b_s = load(brow_b, [1, ME * 128], BF16, "brow_b")
            brow_d_s = load(brow_d, [1, MD * 128], BF16, "brow_d")
            ones_s = wpool.tile([1, B], BF16, tag="ones", name="ones")
            nc.vector.memset(ones_s[:], 1.0)

        # ---------- shared state ----------
        mem_T = big.tile([128, 2 * KE * B * S], BF16, tag="mem_T", name="mem_T")
        memT4 = mem_T[:].rearrange("p (k b s) -> p k b s", k=2 * KE, b=B)
        memN = big.tile([128, B * 2 * KE * 128], BF16, tag="memN", name="memN")

        h_f = [spool.tile([128, KE * B], BF16, tag=f"h_f{j}", name=f"h_f{j}")
               for j in range(2)]
        h_b = [spool.tile([128, KE * B], BF16, tag=f"h_b{j}", name=f"h_b{j}")
               for j in range(2)]
        c_f = spool.tile([128, KE * B], F32, tag="c_f", name="c_f")
        c_b = spool.tile([128, KE * B], F32, tag="c_b", name="c_b")

        try:
            # ============ encoder ============
            # h0 init via outer product + broadcast add
            with tc.tile_pool(name="init_ps", bufs=2, space="PSUM") as ips, \
                 tc.tile_pool(name="init_sb", bufs=2) as isb:
                for d, (hst, dbase) in enumerate([(h_f, 0), (h_b, KE)]):
                    for k in range(KE):
                        ps = ips.tile([128, B], F32, tag="i", name="i")
                        col = (dbase + k) * 128
                        nc.tensor.matmul(ps[:], diff_e_s[:, col:col + 128],
                                         lab_i_s[:, :], start=True, stop=True)
                        f32t = isb.tile([128, B], F32, tag="h0f", name="h0f")
                        nc.vector.tensor_scalar_add(
                            f32t[:], ps[:], e0T_s[:, dbase + k:dbase + k + 1])
                        nc.vector.tensor_copy(hst[0][:, k * B:(k + 1) * B],
                                              f32t[:])
                nc.vector.memset(c_f[:], 0.0)
                nc.vector.memset(c_b[:], 0.0)

            if phases < 2:
                raise _Stop

            # encoder recurrence; bank A = [i f o], bank B = [g]
            GE = KE * 32  # 64
            MH = 3 * ME // 4
            with tc.tile_pool(name="eps0a", bufs=2, space="PSUM") as eps0a, \
                 tc.tile_pool(name="eps0b", bufs=2, space="PSUM") as eps0b, \
                 tc.tile_pool(name="eps1a", bufs=2, space="PSUM") as eps1a, \
                 tc.tile_pool(name="eps1b", bufs=2, space="PSUM") as eps1b, \
                 tc.tile_pool(name="enc_sb", bufs=3) as esb:
                epools = [(eps0a, eps0b), (eps1a, eps1b)]
                for step in range(S):
                    if step == 1:
                        sub_gather(1)
                    elif step == 2:
                        sub_gather(2)
                    elif step == 3:
                        nc.gpsimd.dma_gather(decT[:], tokb[:, :], idx_d_s[:],
                                             NI_D, NI_D, D_EMB, transpose=True,
                                             single_packet=False)
                        nc.vector.tensor_copy(
                            decT[:, 0, 0:B],
                            startT_s[:, 0:1].to_broadcast((128, B)))
                    for d, (hst, cst, wih_s, whh_s, kk0) in enumerate([
                            (h_f, c_f, wih_f_s, whh_f_s, 0),
                            (h_b, c_b, wih_b_s, whh_b_s, KE)]):
                        pa, pb = epools[d]
                        s_in = step if d == 0 else S - 1 - step
                        hcur = hst[step % 2]
                        hnxt = hst[(step + 1) % 2]
                        emb = encT[:, 0, s_in * B:(s_in + 1) * B]
                        psa = pa.tile([128, MH * 32], F32, tag=f"ga{d}",
                                      name=f"ga{d}")
                        psb = pb.tile([128, (ME - MH) * 32], F32, tag=f"gb{d}",
                                      name=f"gb{d}")
                        for mi in range(ME):
                            ps = psa if mi < MH else psb
                            oc = ps[:, (mi % MH) * 32:(mi % MH + 1) * 32]
                            nc.tensor.matmul(
                                oc, wih_s[:, mi * 128:(mi + 1) * 128], emb,
                                start=True, stop=False)
                            for k in range(KE):
                                nc.tensor.matmul(
                                    oc,
                                    whh_s[:, (mi * KE + k) * 128:
                                          (mi * KE + k + 1) * 128],
                                    hcur[:, k * B:(k + 1) * B],
                                    start=False,
                                    stop=(k == KE - 1 and not bias_mode))
                            if bias_mode:
                                bw = brow_f_s if d == 0 else brow_b_s
                                nc.tensor.matmul(
                                    oc, bw[:, mi * 128:(mi + 1) * 128],
                                    ones_s[:, :], start=False, stop=True)
                        # bank A: [i(GE) f(GE) o(GE)]; bank B: [g(GE)]
                        sgifo = esb.tile([128, 3 * GE], F32, tag=f"sg{d}",
                                         name=f"sg{d}")
                        nc.scalar.activation(sgifo[:], psa[:], AF.Sigmoid)
                        gt = esb.tile([128, GE], F32, tag=f"gt{d}",
                                      name=f"gt{d}")
                        nc.scalar.activation(gt[:], psb[:, 0:GE], AF.Tanh)
                        t1 = esb.tile([128, GE], F32, tag=f"t1{d}",
                                      name=f"t1{d}")
                        nc.vector.tensor_tensor(t1[:], sgifo[:, GE:2 * GE],
                                                cst[:], ALU.mult)
                        t2 = esb.tile([128, GE], F32, tag=f"t2{d}",
                                      name=f"t2{d}")
                        nc.vector.tensor_tensor(t2[:], sgifo[:, 0:GE], gt[:],
                                                ALU.mult)
                        nc.vector.tensor_tensor(cst[:], t1[:], t2[:], ALU.add)
                        tct = esb.tile([128, GE], F32, tag=f"tc{d}",
                                       name=f"tc{d}")
                        nc.scalar.activation(tct[:], cst[:], AF.Tanh)
                        nc.vector.tensor_tensor(hnxt[:], sgifo[:, 2 * GE:3 * GE],
                                                tct[:], ALU.mult)
                        s_mem = s_in
                        nc.sync.dma_start(
                            memT4[:, kk0:kk0 + KE, :, s_mem],
                            hnxt[:].rearrange("p (k b) -> p k b", k=KE))

            if phases < 3:
                raise _Stop
            # ============ c_t / h_t decoder init ============
            h_d = [spool.tile([128, KD * B], BF16, tag=f"h_d{j}", name=f"h_d{j}")
                   for j in range(2)]
            c_d = spool.tile([128, KD * B], F32, tag="c_d", name="c_d")
            ccT = spool.tile([128, KD * B], BF16, tag="ccT", name="ccT")
            nc.vector.tensor_copy(ccT[:, 0:KE * B], c_f[:])
            nc.vector.tensor_copy(ccT[:, KE * B:2 * KE * B], c_b[:])
            with tc.tile_pool(name="ct_ps", bufs=2, space="PSUM") as cps, \
                 tc.tile_pool(name="ct_sb", bufs=2) as csb:
                for m in range(KD):
                    ps = cps.tile([128, B], F32, tag="ct", name="ct")
                    for k in range(KD):
                        lt = wtr_s[:, (k * KD + m) * 128:(k * KD + m + 1) * 128]
                        nc.tensor.matmul(ps[:], lt, ccT[:, k * B:(k + 1) * B],
                                         start=(k == 0), stop=(k == KD - 1))
                    # lrelu(y) = 0.55 y + 0.45 |y|
                    ab = csb.tile([128, B], F32, tag="ab", name="ab")
                    nc.scalar.activation(ab[:], ps[:], AF.Abs)
                    idt = csb.tile([128, B], F32, tag="idt", name="idt")
                    nc.scalar.activation(idt[:], ps[:], AF.Identity, scale=0.55)
                    nc.vector.scalar_tensor_tensor(c_d[:, m * B:(m + 1) * B],
                                                   ab[:], 0.45, idt[:],
                                                   ALU.mult, ALU.add)
                for k in range(KD):
                    ps = cps.tile([128, B], F32, tag="ct", name="ct")
                    nc.tensor.matmul(ps[:], diff_s_s[:, k * 128:(k + 1) * 128],
                                     lab_d_s[:, :], start=True, stop=True)
                    f32t = csb.tile([128, B], F32, tag="h0d", name="h0d")
                    nc.vector.tensor_scalar_add(f32t[:], ps[:], s0T_s[:, k:k + 1])
                    nc.vector.tensor_copy(h_d[0][:, k * B:(k + 1) * B], f32t[:])

            if phases < 4:
                raise _Stop
            # ============ decoder + interleaved post-work ============
            GD = KD * 32  # 128
            BLOCKS = [(0, 32), (32, 32), (64, 32), (96, 16), (112, 16)]
            with tc.tile_pool(name="dpsa", bufs=2, space="PSUM") as dpsa, \
                 tc.tile_pool(name="dpsb", bufs=2, space="PSUM") as dpsb, \
                 tc.tile_pool(name="aps", bufs=1, space="PSUM") as apsp, \
                 tc.tile_pool(name="tps", bufs=1, space="PSUM") as tpsp, \
                 tc.tile_pool(name="sps", bufs=2, space="PSUM") as spsp, \
                 tc.tile_pool(name="dec_sb", bufs=3) as dsb, \
                 tc.tile_pool(name="at_sb", bufs=3) as asb, \
                 tc.tile_pool(name="mid_sb", bufs=1) as mlsb, \
                 tc.tile_pool(name="hblk", bufs=2) as hbp, \
                 tc.tile_pool(name="cblk", bufs=1) as cbp, \
                 tc.tile_pool(name="mblk", bufs=1) as mbp, \
                 tc.tile_pool(name="lgp", bufs=2) as lgp:

                hblk_t = {}
                cblk_t = {}
                mblk_t = {}
                pT_t = {}

                def unit_memN(b):
                    def emit():
                        for k in range(2 * KE):
                            mn_ps = tpsp.tile([128, 128], BF16, tag="tp",
                                              name="mnp", padded_shape=[128, 128])
                            nc.tensor.transpose(mn_ps[:], memT4[:, k, b, :],
                                                ident[:])
                            nc.vector.tensor_copy(
                                memN[:, (b * 2 * KE + k) * 128:
                                     (b * 2 * KE + k + 1) * 128], mn_ps[:])
                    return emit

                def unit_score(tau, bg):
                    def emit():
                        t0, Tcb = BLOCKS[tau]
                        Hb = hblk_t[tau]
                        H4 = Hb[:].rearrange("p (k b t) -> p k b t", k=KD, b=B)
                        b0 = bg * BG
                        a_ps = apsp.tile([128, S], F32, tag="a", name="a")
                        for j in range(BG):
                            for k in range(KD):
                                nc.tensor.matmul(
                                    a_ps[32 * j:32 * j + Tcb, :],
                                    H4[:, k, b0 + j, :], memT4[:, k, b0 + j, :],
                                    start=(k == 0), stop=(k == KD - 1),
                                    tile_position=(0, 32 * j))
                        mx = asb.tile([128, 1], F32, tag="mx", name="mx")
                        nc.vector.tensor_reduce(mx[:], a_ps[:], AX.X, ALU.max,
                                                negate=True)
                        mx2 = asb.tile([128, 1], F32, tag="mx2", name="mx2")
                        nc.scalar.mul(mx2[:], mx[:], SCALE)
                        ex = asb.tile([128, S], F32, tag="ex", name="ex")
                        den = asb.tile([128, 1], F32, tag="den", name="den")
                        nc.scalar.activation(ex[:], a_ps[:], AF.Exp, bias=mx2[:],
                                             scale=SCALE, accum_out=den[:])
                        rec = asb.tile([128, 1], F32, tag="rec", name="rec")
                        nc.vector.reciprocal(rec[:], den[:])
                        p_sb = asb.tile([128, S], BF16, tag="p", name="p")
                        nc.vector.tensor_scalar_mul(p_sb[:], ex[:], rec[:])
                        pt_ps = tpsp.tile([128, 128], BF16, tag="tp", name="pt",
                                          padded_shape=[128, 128])
                        nc.tensor.transpose(pt_ps[:], p_sb[:], ident[:])
                        pT = asb.tile([128, 128], BF16, tag="pT", name="pT")
                        nc.vector.tensor_copy(pT[:], pt_ps[:])
                        pT_t[(tau, bg)] = pT
                    return emit

                def unit_ctx(tau, bg):
                    def emit():
                        t0, Tcb = BLOCKS[tau]
                        b0 = bg * BG
                        pT = pT_t.pop((tau, bg))
                        Cb = cblk_t[tau]
                        for j in range(BG):
                            b = b0 + j
                            for k in range(KD):
                                c_ps = spsp.tile([128, 512], F32, tag="sp",
                                                 name="cx")
                                nc.tensor.matmul(
                                    c_ps[:, 0:Tcb],
                                    memN[:, (b * 2 * KE + k) * 128:
                                         (b * 2 * KE + k + 1) * 128],
                                    pT[:, 32 * j:32 * j + Tcb],
                                    start=True, stop=True)
                                nc.scalar.copy(
                                    Cb[:, k, b * Tcb:(b + 1) * Tcb],
                                    c_ps[:, 0:Tcb])
                    return emit

                def unit_mid(tau, half, m):
                    def emit():
                        t0, Tcb = BLOCKS[tau]
                        Hb = hblk_t[tau]
                        Hf = Hb[:].rearrange("p (k bt) -> p k bt", k=KD)
                        Cb = cblk_t[tau]
                        Mb = mblk_t[tau]
                        ps = spsp.tile([128, 512], F32, tag="sp", name="md")
                        c0 = half * 512
                        for k in range(KD):
                            lt = wf1_s[:, (k * KD + m) * 128:
                                       (k * KD + m + 1) * 128]
                            nc.tensor.matmul(ps[:], lt, Hf[:, k, c0:c0 + 512],
                                             start=(k == 0), stop=False)
                        for k in range(KD):
                            kk = KD + k
                            lt = wf1_s[:, (kk * KD + m) * 128:
                                       (kk * KD + m + 1) * 128]
                            nc.tensor.matmul(ps[:], lt, Cb[:, k, c0:c0 + 512],
                                             start=False, stop=(k == KD - 1))
                        ab = mlsb.tile([128, 512], F32, tag="mab", name="mab")
                        nc.scalar.activation(ab[:], ps[:], AF.Abs,
                                             bias=b1a_s[:, m:m + 1])
                        idt = mlsb.tile([128, 512], F32, tag="mid", name="mid")
                        nc.scalar.activation(idt[:], ps[:], AF.Identity,
                                             scale=0.55, bias=b1h_s[:, m:m + 1])
                        nc.vector.scalar_tensor_tensor(
                            Mb[:, m, c0:c0 + 512], ab[:], 0.45, idt[:],
                            ALU.mult, ALU.add)
                    return emit

                def unit_vocab(tau, btm, hf):
                    def emit():
                        t0, Tcb = BLOCKS[tau]
                        nb = 128 // Tcb
                        Mb = mblk_t[tau]
                        wf2v = wf2_s[:].rearrange("p (k v) -> p k v", k=KD)
                        b0 = btm * nb
                        HV = VS // 2
                        lgrow = lgp.tile([128, HV], BF16, tag="lgr",
                                         name="lgr")
                        for cch in range(HV // 500):
                            w0 = hf * HV + cch * 500
                            lg = spsp.tile([128, 512], F32, tag="sp",
                                           name="lg")
                            for k in range(KD):
                                nc.tensor.matmul(
                                    lg[:, 0:500],
                                    Mb[:, k, btm * 128:(btm + 1) * 128],
                                    wf2v[:, k, w0:w0 + 500],
                                    start=(k == 0), stop=(k == KD - 1))
                            nc.vector.tensor_copy(
                                lgrow[:, cch * 500:(cch + 1) * 500],
                                lg[:, 0:500])
                        for j in range(nb):
                            nc.sync.dma_start(
                                out.ap()[b0 + j, t0:t0 + Tcb,
                                         hf * HV:(hf + 1) * HV],
                                lgrow[Tcb * j:Tcb * (j + 1), :])
                    return emit

                MHD = 3 * MD // 4

                def block_units(tau):
                    t0, Tcb = BLOCKS[tau]
                    u = []
                    for bg in range(B // BG):
                        u.append(unit_score(tau, bg))
                        u.append(unit_ctx(tau, bg))
                    for half in range(B * Tcb // 512):
                        for m in range(KD):
                            u.append(unit_mid(tau, half, m))
                    for btm in range(B * Tcb // 128):
                        for hf in range(2):
                            u.append(unit_vocab(tau, btm, hf))
                    return u

                def alloc_blk(tau):
                    t0, Tcb = BLOCKS[tau]
                    cblk_t[tau] = cbp.tile([128, KD, B * Tcb], BF16,
                                           tag="cb", name=f"cb{tau}")
                    if phases >= 6:
                        mblk_t[tau] = mbp.tile([128, KD, B * Tcb], BF16,
                                               tag="mb", name=f"mb{tau}")

                queue = [unit_memN(b) for b in range(B)] if phases >= 5 else []
                for tau, (t0, Tcb) in enumerate(BLOCKS):
                    hblk_t[tau] = hbp.tile([128, KD * B * Tcb], BF16,
                                           tag="hb", name=f"hb{tau}")
                    if tau >= 1 and phases >= 5:
                        alloc_blk(tau - 1)
                        queue.extend(block_units(tau - 1) if phases >= 6 else
                                     [unit_score(tau - 1, bg)
                                      for bg in range(B // BG)])
                    for trel in range(Tcb):
                        t = t0 + trel
                        hcur = h_d[t % 2]
                        hnxt = h_d[(t + 1) % 2]
                        emb = decT[:, 0, t * B:(t + 1) * B]
                        psa = dpsa.tile([128, MHD * 32], F32, tag="gda",
                                        name="gda")
                        psb = dpsb.tile([128, (MD - MHD) * 32], F32, tag="gdb",
                                        name="gdb")
                        for mi in range(MD):
                            ps = psa if mi < MHD else psb
                            oc = ps[:, (mi % MHD) * 32:(mi % MHD + 1) * 32]
                            nc.tensor.matmul(
                                oc, wih_d_s[:, mi * 128:(mi + 1) * 128],
                                emb, start=True, stop=False)
                            for k in range(KD):
                                nc.tensor.matmul(
                                    oc,
                                    whh_d_s[:, (mi * KD + k) * 128:
                                            (mi * KD + k + 1) * 128],
                                    hcur[:, k * B:(k + 1) * B],
                                    start=False,
                                    stop=(k == KD - 1 and not bias_mode))
                            if bias_mode:
                                nc.tensor.matmul(
                                    oc, brow_d_s[:, mi * 128:(mi + 1) * 128],
                                    ones_s[:, :], start=False, stop=True)
                        # bank A: [i f o] in tanh form (sigmoid via
                        # tanh(x/2); factor-2s folded into host weights:
                        # state is cc=2c, h2=2h); bank B: [g]
                        th = dsb.tile([128, 3 * GD], F32, tag="sgd", name="sgd")
                        nc.scalar.activation(th[:], psa[:], AF.Tanh, scale=0.25)
                        gt = dsb.tile([128, GD], F32, tag="gtd", name="gtd")
                        nc.scalar.activation(gt[:], psb[:, 0:GD], AF.Tanh,
                                             scale=0.5)
                        t1 = dsb.tile([128, GD], F32, tag="t1d", name="t1d")
                        nc.vector.scalar_tensor_tensor(
                            t1[:], th[:, GD:2 * GD], 1.0, c_d[:],
                            ALU.add, ALU.mult)
                        t2 = dsb.tile([128, GD], F32, tag="t2d", name="t2d")
                        nc.vector.scalar_tensor_tensor(
                            t2[:], th[:, 0:GD], 1.0, gt[:], ALU.add, ALU.mult)
                        nc.vector.scalar_tensor_tensor(
                            c_d[:], t1[:], 0.5, t2[:], ALU.mult, ALU.add)
                        tct = dsb.tile([128, GD], F32, tag="tcd", name="tcd")
                        nc.scalar.activation(tct[:], c_d[:], AF.Tanh, scale=0.5)
                        nc.vector.scalar_tensor_tensor(
                            hnxt[:], th[:, 2 * GD:3 * GD], 1.0, tct[:],
                            ALU.add, ALU.mult)
                        Hb4 = hblk_t[tau][:].rearrange("p (k b t) -> p k b t",
                                                       k=KD, b=B)
                        nc.vector.tensor_copy(
                            Hb4[:, :, :, trel],
                            hnxt[:].rearrange("p (k b) -> p k b", k=KD))
                        # paced interleave of previous block's post-work
                        rem_steps = Tcb - trel
                        npop = (len(queue) + rem_steps - 1) // rem_steps
                        for _ in range(npop):
                            queue.pop(0)()

                # tail: last block's post-work
                if phases >= 5:
                    tau = len(BLOCKS) - 1
                    alloc_blk(tau)
                    for u in (block_units(tau) if phases >= 6 else
                              [unit_score(tau, bg) for bg in range(B // BG)]):
                        u()
        except _Stop:
            pass
    nc.compile()
    return nc


def prep_inputs(i, S=128, T=128, V=32000, VS=4000, fp8=False):
    def as_np(x, dt=np.float32):
        return np.ascontiguousarray(np.asarray(x), dtype=dt)

    whh_np = f8e4 if fp8 else bf16
    tok = as_np(i["tok_emb"]).astype(bf16)

    def idx_prep(flat):
        a = flat.astype(np.int16).reshape(-1, 16).T
        return np.ascontiguousarray(np.tile(a, (8, 1)))

    inp = as_np(i["inp"], np.int64)
    x = as_np(i["x"], np.int64)
    idx_e = idx_prep(inp.T.reshape(-1))
    dmat = np.zeros((B, T), np.int64)
    dmat[:, 1:] = x[:, :T - 1]
    idx_d = idx_prep(dmat.T.reshape(-1))

    startT = as_np(i["start_emb"]).reshape(D_EMB, 1).astype(bf16)
    est = as_np(i["enc_style_emb"])
    diff_e = (est[1] - est[0]).reshape(1, -1).astype(bf16)
    e0T = np.ascontiguousarray(est[0].reshape(KD, 128).T)
    sty = 2.0 * as_np(i["style_emb"])  # decoder h-state kept as 2h
    diff_s = (sty[1] - sty[0]).reshape(1, -1).astype(bf16)
    s0T = np.ascontiguousarray(sty[0].reshape(KD, 128).T)
    lab_i = as_np(i["label_i"], np.float32).reshape(1, B).astype(bf16)
    lab_d = as_np(i["label"], np.float32).reshape(1, B).astype(bf16)

    def wihP(w, nm, perm):
        # w [4H, 128] -> [128, nm*128], tile mi = chunk perm[mi], lhsT layout
        a = w.reshape(nm, 128, 128)          # [m, out, in]
        a = a[perm]                          # permuted
        return np.ascontiguousarray(a.transpose(2, 0, 1).reshape(128, nm * 128)
                                    ).astype(bf16)

    def whhP(w, nk, nm, perm):
        # w [4H, H] -> [128, nm*nk*128], (m-perm, k)-major
        a = w.reshape(nm, 128, nk, 128)      # [m, out, k, in]
        a = a[perm]
        a = a.transpose(3, 0, 2, 1)          # [in, m, k, out]
        return np.ascontiguousarray(a.reshape(128, nm * nk * 128)).astype(whh_np)

    def whhT(w, nk, nm):
        # (k, m)-major, unpermuted (for wtr / wf1)
        a = w.reshape(nm, 128, nk, 128)
        a = a.transpose(3, 2, 0, 1)
        return np.ascontiguousarray(a.reshape(128, nk * nm * 128)).astype(bf16)

    wih_f = wihP(as_np(i["Wih_f"]), ME, PERM_E)
    wih_b = wihP(as_np(i["Wih_b"]), ME, PERM_E)
    wih_d = wihP(2.0 * as_np(i["Wih_d"]), MD, PERM_D)
    whh_f = whhP(as_np(i["Whh_f"]), KE, ME, PERM_E)
    whh_b = whhP(as_np(i["Whh_b"]), KE, ME, PERM_E)
    whh_d = whhP(as_np(i["Whh_d"]), KD, MD, PERM_D)
    wtr = whhT(2.0 * as_np(i["W_tr"]), KD, KD)
    wf1_np = as_np(i["W_f1"]).copy()
    wf1_np[:, :D_DEC] *= 0.5          # h-input half reads h2 = 2h
    wf1 = whhT(wf1_np, 8, KD)
    wf2_full = as_np(i["W_f2"])
    b1 = as_np(i["b_f1"])
    b1a = np.ascontiguousarray(b1.reshape(KD, 128).T)
    b1h = np.ascontiguousarray((0.55 * b1).reshape(KD, 128).T)

    bs_f = as_np(i["bih_f"]) + as_np(i["bhh_f"])
    bs_b = as_np(i["bih_b"]) + as_np(i["bhh_b"])
    bs_d = as_np(i["bih_d"]) + as_np(i["bhh_d"])
    bias_mode = bool(np.any(bs_f) or np.any(bs_b) or np.any(bs_d))

    common = dict(tokb=tok, idx_e=idx_e, idx_d=idx_d, startT=startT,
                  diff_e=diff_e, e0T=e0T, lab_i=lab_i,
                  diff_s=diff_s, s0T=s0T, lab_d=lab_d,
                  wih_f=wih_f, wih_b=wih_b, wih_d=wih_d,
                  whh_f=whh_f, whh_b=whh_b, whh_d=whh_d,
                  wtr=wtr, wf1=wf1, b1a=b1a, b1h=b1h)
    if bias_mode:
        def brow(v, nm, perm):
            a = v.reshape(nm, 128)[perm]
            return np.ascontiguousarray(a.reshape(1, nm * 128)).astype(bf16)
        common.update(brow_f=brow(bs_f, ME, PERM_E), brow_b=brow(bs_b, ME, PERM_E),
                      brow_d=brow(2.0 * bs_d, MD, PERM_D))

    in_maps = []
    for c in range(N_CORES):
        shard = wf2_full[c * VS:(c + 1) * VS]
        a = shard.reshape(VS, KD, 128)
        wf2 = np.ascontiguousarray(a.transpose(2, 1, 0).reshape(128, KD * VS)
                                   ).astype(bf16)
        in_maps.append(dict(common, wf2=wf2))
    return in_maps, bias_mode


_NC_CACHE = {}
_FP8 = True


def kernel(**inputs):
    in_maps, bias_mode = prep_inputs(inputs, fp8=_FP8)
    key = (bias_mode, _FP8)
    if key not in _NC_CACHE:
        _NC_CACHE[key] = build(fp8=_FP8, bias_mode=bias_mode)
    nc = _NC_CACHE[key]
    res = run_bass_kernel_spmd(nc, in_maps, core_ids=list(range(N_CORES)))
    return np.concatenate([r["out"].astype(np.float32) for r in res.results],
                          axis=2)


# revision 32
# speedup vs baseline: 1.9309x; 1.9309x over previous
"""DenoiseLSTM Trainium2 kernel (8 NeuronCores, SPMD) — v2.

Strategy vs v1 baseline:
- Input projections are folded into the recurrent loops as extra matmul
  accumulation steps (embeddings live in SBUF from the gather), killing the
  DMA-scatter projection phase and the per-step g=ps+xt vector add.
- Gate m-chunks are permuted host-side to [i, f, o, g] so one sigmoid
  activation covers i/f/o straight out of PSUM; tanh covers g.
- Attention + FFN + vocab projection are restructured per 32-step t-block
  and emitted interleaved with decoder steps, so the Tile scheduler fills
  the decoder's dependency-stall gaps with post-work.
- Attention scores pack 4 batch rows into one [128,S] PSUM tile via
  tile_position column packing; softmax runs on full 128 partitions.
- Output is written bf16 and upcast on host (halves output DMA).
- Optional: recurrent Whh weights in fp8-e4m3 (halves LDWEIGHTS traffic);
  CPU-simulated rel err 0.0079 vs 2e-2 budget.

Replicated recurrences on all 8 cores; vocab projection sharded over V
(4000 columns per core); host concatenates V-shards.
"""
import sys

sys.path.insert(0, "/opt/trn_rl_repo")

from contextlib import ExitStack

import numpy as np
import ml_dtypes

import concourse.bass as bass
import concourse.bacc as bacc
import concourse.mybir as mybir
import concourse.tile as tile
from concourse.bass_utils import run_bass_kernel_spmd
from concourse.masks import make_identity

bf16 = ml_dtypes.bfloat16
f8e4 = ml_dtypes.float8_e4m3
F32 = mybir.dt.float32
BF16 = mybir.dt.bfloat16
FP8E4 = mybir.dt.float8e4
I16 = mybir.dt.int16
AF = mybir.ActivationFunctionType
ALU = mybir.AluOpType
AX = mybir.AxisListType

B = 32
D_EMB = 128
D_ENC = 256
D_DEC = 512
N_CORES = 8
KE = D_ENC // 128   # 2
KD = D_DEC // 128   # 4
ME = 4 * D_ENC // 128  # 8 gate chunks (encoder)
MD = 4 * D_DEC // 128  # 16 gate chunks (decoder)
# m-chunk permutation: [i-chunks, f-chunks, o-chunks, g-chunks]
PERM_E = [0, 1, 2, 3, 6, 7, 4, 5]
PERM_D = [0, 1, 2, 3, 4, 5, 6, 7, 12, 13, 14, 15, 8, 9, 10, 11]


class _Stop(Exception):
    pass


def build(S=128, T=128, V=32000, VS=4000, phases=6, fp8=False, bias_mode=False):
    NI_E = B * S
    NI_D = B * T
    Tc = 32                      # timesteps per post-work block
    NBLK = T // Tc               # 4
    BG = 4                       # batch rows packed per score tile
    SCALE = 0.5 / float(np.sqrt(np.float32(2 * D_ENC)))
    WHH_DT = FP8E4 if fp8 else BF16

    nc = bacc.Bacc("TRN2", target_bir_lowering=False, debug=False)

    # ---- external inputs ----
    tokb = nc.dram_tensor("tokb", [V, D_EMB], BF16, kind="ExternalInput")
    idx_e = nc.dram_tensor("idx_e", [128, NI_E // 16], I16, kind="ExternalInput")
    idx_d = nc.dram_tensor("idx_d", [128, NI_D // 16], I16, kind="ExternalInput")
    startT = nc.dram_tensor("startT", [128, 1], BF16, kind="ExternalInput")
    diff_e = nc.dram_tensor("diff_e", [1, 2 * D_ENC], BF16, kind="ExternalInput")
    e0T = nc.dram_tensor("e0T", [128, KD], F32, kind="ExternalInput")
    lab_i = nc.dram_tensor("lab_i", [1, B], BF16, kind="ExternalInput")
    diff_s = nc.dram_tensor("diff_s", [1, D_DEC], BF16, kind="ExternalInput")
    s0T = nc.dram_tensor("s0T", [128, KD], F32, kind="ExternalInput")
    lab_d = nc.dram_tensor("lab_d", [1, B], BF16, kind="ExternalInput")
    # input-projection weights (bf16), m-perm-major
    wih_f = nc.dram_tensor("wih_f", [128, ME * 128], BF16, kind="ExternalInput")
    wih_b = nc.dram_tensor("wih_b", [128, ME * 128], BF16, kind="ExternalInput")
    wih_d = nc.dram_tensor("wih_d", [128, MD * 128], BF16, kind="ExternalInput")
    # recurrent weights, (m-perm, k)-major, optionally fp8
    whh_f = nc.dram_tensor("whh_f", [128, ME * KE * 128], WHH_DT, kind="ExternalInput")
    whh_b = nc.dram_tensor("whh_b", [128, ME * KE * 128], WHH_DT, kind="ExternalInput")
    whh_d = nc.dram_tensor("whh_d", [128, MD * KD * 128], WHH_DT, kind="ExternalInput")
    wtr = nc.dram_tensor("wtr", [128, KD * KD * 128], BF16, kind="ExternalInput")
    wf1 = nc.dram_tensor("wf1", [128, 8 * KD * 128], BF16, kind="ExternalInput")
    wf2 = nc.dram_tensor("wf2", [128, KD * VS], BF16, kind="ExternalInput")
    b1a = nc.dram_tensor("b1a", [128, KD], F32, kind="ExternalInput")
    b1h = nc.dram_tensor("b1h", [128, KD], F32, kind="ExternalInput")
    if bias_mode:
        brow_f = nc.dram_tensor("brow_f", [1, ME * 128], BF16, kind="ExternalInput")
        brow_b = nc.dram_tensor("brow_b", [1, ME * 128], BF16, kind="ExternalInput")
        brow_d = nc.dram_tensor("brow_d", [1, MD * 128], BF16, kind="ExternalInput")

    out = nc.dram_tensor("out", [B, T, VS], BF16, kind="ExternalOutput")

    with tile.TileContext(nc) as tc, ExitStack() as ctx:
        wpool = ctx.enter_context(tc.tile_pool(name="weights", bufs=1))
        spool = ctx.enter_context(tc.tile_pool(name="state", bufs=1))
        big = ctx.enter_context(tc.tile_pool(name="big", bufs=1))

        def load(dram, shape, dtype, tag):
            t = wpool.tile(shape, dtype, tag=tag, name=tag)
            nc.sync.dma_start(t[:], dram[:, :])
            return t

        # gathers first: idx DMAs + encoder-token gather start immediately,
        # overlapping the weight streams
        idx_e_s = wpool.tile([128, NI_E // 16], I16, tag="idx_e", name="idx_e")
        nc.sync.dma_start(idx_e_s[:], idx_e[:, :])
        idx_d_s = wpool.tile([128, NI_D // 16], I16, tag="idx_d", name="idx_d")
        nc.sync.dma_start(idx_d_s[:], idx_d[:, :])
        encT = big.tile([128, 1, NI_E], BF16, tag="encT", name="encT")
        NQ = NI_E // 4

        def sub_gather(gc):
            nc.gpsimd.dma_gather(encT[:, :, gc * NQ:(gc + 1) * NQ], tokb[:, :],
                                 idx_e_s[:, gc * NQ // 16:(gc + 1) * NQ // 16],
                                 NQ, NQ, D_EMB, transpose=True,
                                 single_packet=False)

        sub_gather(0)
        sub_gather(3)
        decT = big.tile([128, 1, NI_D], BF16, tag="decT", name="decT")

        wih_f_s = load(wih_f, [128, ME * 128], BF16, "wih_f")
        wih_b_s = load(wih_b, [128, ME * 128], BF16, "wih_b")
        wih_d_s = load(wih_d, [128, MD * 128], BF16, "wih_d")
        whh_f_s = load(whh_f, [128, ME * KE * 128], WHH_DT, "whh_f")
        whh_b_s = load(whh_b, [128, ME * KE * 128], WHH_DT, "whh_b")
        whh_d_s = load(whh_d, [128, MD * KD * 128], WHH_DT, "whh_d")
        wtr_s = load(wtr, [128, KD * KD * 128], BF16, "wtr")
        wf1_s = load(wf1, [128, 8 * KD * 128], BF16, "wf1")
        wf2_s = load(wf2, [128, KD * VS], BF16, "wf2")
        b1a_s = load(b1a, [128, KD], F32, "b1a")
        b1h_s = load(b1h, [128, KD], F32, "b1h")
        e0T_s = load(e0T, [128, KD], F32, "e0T")
        s0T_s = load(s0T, [128, KD], F32, "s0T")
        startT_s = load(startT, [128, 1], BF16, "startT")
        ident = wpool.tile([128, 128], BF16, tag="ident", name="ident")
        make_identity(nc, ident)

        diff_e_s = wpool.tile([1, 2 * D_ENC], BF16, tag="diff_e", name="diff_e")
        nc.sync.dma_start(diff_e_s[:], diff_e[:, :])
        diff_s_s = wpool.tile([1, D_DEC], BF16, tag="diff_s", name="diff_s")
        nc.sync.dma_start(diff_s_s[:], diff_s[:, :])
        lab_i_s = wpool.tile([1, B], BF16, tag="lab_i", name="lab_i")
        nc.sync.dma_start(lab_i_s[:], lab_i[:, :])
        lab_d_s = wpool.tile([1, B], BF16, tag="lab_d", name="lab_d")
        nc.sync.dma_start(lab_d_s[:], lab_d[:, :])
        if bias_mode:
            brow_f_s = load(brow_f, [1, ME * 128], BF16, "brow_f")
            brow_b_s = load(brow_b, [1, ME * 128], BF16, "brow_b")
            brow_d_s = load(brow_d, [1, MD * 128], BF16, "brow_d")
            ones_s = wpool.tile([1, B], BF16, tag="ones", name="ones")
            nc.vector.memset(ones_s[:], 1.0)

        # ---------- shared state ----------
        mem_T = big.tile([128, 2 * KE * B * S], BF16, tag="mem_T", name="mem_T")
        memT4 = mem_T[:].rearrange("p (k b s) -> p k b s", k=2 * KE, b=B)
        memN = big.tile([128, B * 2 * KE * 128], BF16, tag="memN", name="memN")

        h_f = [spool.tile([128, KE * B], BF16, tag=f"h_f{j}", name=f"h_f{j}")
               for j in range(2)]
        h_b = [spool.tile([128, KE * B], BF16, tag=f"h_b{j}", name=f"h_b{j}")
               for j in range(2)]
        c_f = spool.tile([128, KE * B], F32, tag="c_f", name="c_f")
        c_b = spool.tile([128, KE * B], F32, tag="c_b", name="c_b")

        try:
            # ============ encoder ============
            # h0 init via outer product + broadcast add
            with tc.tile_pool(name="init_ps", bufs=2, space="PSUM") as ips, \
                 tc.tile_pool(name="init_sb", bufs=2) as isb:
                for d, (hst, dbase) in enumerate([(h_f, 0), (h_b, KE)]):
                    for k in range(KE):
                        ps = ips.tile([128, B], F32, tag="i", name="i")
                        col = (dbase + k) * 128
                        nc.tensor.matmul(ps[:], diff_e_s[:, col:col + 128],
                                         lab_i_s[:, :], start=True, stop=True)
                        f32t = isb.tile([128, B], F32, tag="h0f", name="h0f")
                        nc.vector.tensor_scalar_add(
                            f32t[:], ps[:], e0T_s[:, dbase + k:dbase + k + 1])
                        nc.vector.tensor_copy(hst[0][:, k * B:(k + 1) * B],
                                              f32t[:])
                nc.vector.memset(c_f[:], 0.0)
                nc.vector.memset(c_b[:], 0.0)

            if phases < 2:
                raise _Stop

            # encoder recurrence; bank A = [i f o], bank B = [g]
            GE = KE * 32  # 64
            MH = 3 * ME // 4
            with tc.tile_pool(name="eps0a", bufs=2, space="PSUM") as eps0a, \
                 tc.tile_pool(name="eps0b", bufs=2, space="PSUM") as eps0b, \
                 tc.tile_pool(name="eps1a", bufs=2, space="PSUM") as eps1a, \
                 tc.tile_pool(name="eps1b", bufs=2, space="PSUM") as eps1b, \
                 tc.tile_pool(name="enc_sb", bufs=3) as esb:
                epools = [(eps0a, eps0b), (eps1a, eps1b)]
                for step in range(S):
                    if step == 1:
                        sub_gather(1)
                    elif step == 2:
                        sub_gather(2)
                    elif step == 3:
                        nc.gpsimd.dma_gather(decT[:], tokb[:, :], idx_d_s[:],
                                             NI_D, NI_D, D_EMB, transpose=True,
                                             single_packet=False)
                        nc.vector.tensor_copy(
                            decT[:, 0, 0:B],
                            startT_s[:, 0:1].to_broadcast((128, B)))
                    for d, (hst, cst, wih_s, whh_s, kk0) in enumerate([
                            (h_f, c_f, wih_f_s, whh_f_s, 0),
                            (h_b, c_b, wih_b_s, whh_b_s, KE)]):
                        pa, pb = epools[d]
                        s_in = step if d == 0 else S - 1 - step
                        hcur = hst[step % 2]
                        hnxt = hst[(step + 1) % 2]
                        emb = encT[:, 0, s_in * B:(s_in + 1) * B]
                        psa = pa.tile([128, MH * 32], F32, tag=f"ga{d}",
                                      name=f"ga{d}")
                        psb = pb.tile([128, (ME - MH) * 32], F32, tag=f"gb{d}",
                                      name=f"gb{d}")
                        for mi in range(ME):
                            ps = psa if mi < MH else psb
                            oc = ps[:, (mi % MH) * 32:(mi % MH + 1) * 32]
                            nc.tensor.matmul(
                                oc, wih_s[:, mi * 128:(mi + 1) * 128], emb,
                                start=True, stop=False)
                            for k in range(KE):
                                nc.tensor.matmul(
                                    oc,
                                    whh_s[:, (mi * KE + k) * 128:
                                          (mi * KE + k + 1) * 128],
                                    hcur[:, k * B:(k + 1) * B],
                                    start=False,
                                    stop=(k == KE - 1 and not bias_mode))
                            if bias_mode:
                                bw = brow_f_s if d == 0 else brow_b_s
                                nc.tensor.matmul(
                                    oc, bw[:, mi * 128:(mi + 1) * 128],
                                    ones_s[:, :], start=False, stop=True)
                        # bank A: [i(GE) f(GE) o(GE)]; bank B: [g(GE)]
                        sgifo = esb.tile([128, 3 * GE], F32, tag=f"sg{d}",
                                         name=f"sg{d}")
                        nc.scalar.activation(sgifo[:], psa[:], AF.Sigmoid)
                        gt = esb.tile([128, GE], F32, tag=f"gt{d}",
                                      name=f"gt{d}")
                        nc.scalar.activation(gt[:], psb[:, 0:GE], AF.Tanh)
                        t1 = esb.tile([128, GE], F32, tag=f"t1{d}",
                                      name=f"t1{d}")
                        nc.vector.tensor_tensor(t1[:], sgifo[:, GE:2 * GE],
                                                cst[:], ALU.mult)
                        t2 = esb.tile([128, GE], F32, tag=f"t2{d}",
                                      name=f"t2{d}")
                        nc.vector.tensor_tensor(t2[:], sgifo[:, 0:GE], gt[:],
                                                ALU.mult)
                        nc.vector.tensor_tensor(cst[:], t1[:], t2[:], ALU.add)
                        tct = esb.tile([128, GE], F32, tag=f"tc{d}",
                                       name=f"tc{d}")
                        nc.scalar.activation(tct[:], cst[:], AF.Tanh)
                        nc.vector.tensor_tensor(hnxt[:], sgifo[:, 2 * GE:3 * GE],
                                                tct[:], ALU.mult)
                        s_mem = s_in
                        nc.vector.tensor_copy(
                            memT4[:, kk0:kk0 + KE, :, s_mem],
                            hnxt[:].rearrange("p (k b) -> p k b", k=KE))

            if phases < 3:
                raise _Stop
            # ============ c_t / h_t decoder init ============
            h_d = [spool.tile([128, KD * B], BF16, tag=f"h_d{j}", name=f"h_d{j}")
                   for j in range(2)]
            c_d = spool.tile([128, KD * B], F32, tag="c_d", name="c_d")
            ccT = spool.tile([128, KD * B], BF16, tag="ccT", name="ccT")
            nc.vector.tensor_copy(ccT[:, 0:KE * B], c_f[:])
            nc.vector.tensor_copy(ccT[:, KE * B:2 * KE * B], c_b[:])
            with tc.tile_pool(name="ct_ps", bufs=2, space="PSUM") as cps, \
                 tc.tile_pool(name="ct_sb", bufs=2) as csb:
                for m in range(KD):
                    ps = cps.tile([128, B], F32, tag="ct", name="ct")
                    for k in range(KD):
                        lt = wtr_s[:, (k * KD + m) * 128:(k * KD + m + 1) * 128]
                        nc.tensor.matmul(ps[:], lt, ccT[:, k * B:(k + 1) * B],
                                         start=(k == 0), stop=(k == KD - 1))
                    # lrelu(y) = 0.55 y + 0.45 |y|
                    ab = csb.tile([128, B], F32, tag="ab", name="ab")
                    nc.scalar.activation(ab[:], ps[:], AF.Abs)
                    idt = csb.tile([128, B], F32, tag="idt", name="idt")
                    nc.scalar.activation(idt[:], ps[:], AF.Identity, scale=0.55)
                    nc.vector.scalar_tensor_tensor(c_d[:, m * B:(m + 1) * B],
                                                   ab[:], 0.45, idt[:],
                                                   ALU.mult, ALU.add)
                for k in range(KD):
                    ps = cps.tile([128, B], F32, tag="ct", name="ct")
                    nc.tensor.matmul(ps[:], diff_s_s[:, k * 128:(k + 1) * 128],
                                     lab_d_s[:, :], start=True, stop=True)
                    f32t = csb.tile([128, B], F32, tag="h0d", name="h0d")
                    nc.vector.tensor_scalar_add(f32t[:], ps[:], s0T_s[:, k:k + 1])
                    nc.vector.tensor_copy(h_d[0][:, k * B:(k + 1) * B], f32t[:])

            if phases < 4:
                raise _Stop
            # ============ decoder + interleaved post-work ============
            GD = KD * 32  # 128
            BLOCKS = [(0, 32), (32, 32), (64, 32), (96, 16), (112, 16)]
            with tc.tile_pool(name="dpsa", bufs=2, space="PSUM") as dpsa, \
                 tc.tile_pool(name="dpsb", bufs=2, space="PSUM") as dpsb, \
                 tc.tile_pool(name="aps", bufs=1, space="PSUM") as apsp, \
                 tc.tile_pool(name="tps", bufs=1, space="PSUM") as tpsp, \
                 tc.tile_pool(name="sps", bufs=2, space="PSUM") as spsp, \
                 tc.tile_pool(name="dec_sb", bufs=3) as dsb, \
                 tc.tile_pool(name="at_sb", bufs=3) as asb, \
                 tc.tile_pool(name="mid_sb", bufs=1) as mlsb, \
                 tc.tile_pool(name="hblk", bufs=2) as hbp, \
                 tc.tile_pool(name="cblk", bufs=1) as cbp, \
                 tc.tile_pool(name="mblk", bufs=1) as mbp, \
                 tc.tile_pool(name="lgp", bufs=2) as lgp:

                hblk_t = {}
                cblk_t = {}
                mblk_t = {}
                pT_t = {}

                def unit_memN(b):
                    def emit():
                        for k in range(2 * KE):
                            mn_ps = tpsp.tile([128, 128], BF16, tag="tp",
                                              name="mnp", padded_shape=[128, 128])
                            nc.tensor.transpose(mn_ps[:], memT4[:, k, b, :],
                                                ident[:])
                            nc.vector.tensor_copy(
                                memN[:, (b * 2 * KE + k) * 128:
                                     (b * 2 * KE + k + 1) * 128], mn_ps[:])
                    return emit

                def unit_score(tau, bg):
                    def emit():
                        t0, Tcb = BLOCKS[tau]
                        Hb = hblk_t[tau]
                        H4 = Hb[:].rearrange("p (k b t) -> p k b t", k=KD, b=B)
                        b0 = bg * BG
                        a_ps = apsp.tile([128, S], F32, tag="a", name="a")
                        for j in range(BG):
                            for k in range(KD):
                                nc.tensor.matmul(
                                    a_ps[32 * j:32 * j + Tcb, :],
                                    H4[:, k, b0 + j, :], memT4[:, k, b0 + j, :],
                                    start=(k == 0), stop=(k == KD - 1),
                                    tile_position=(0, 32 * j))
                        mx = asb.tile([128, 1], F32, tag="mx", name="mx")
                        nc.vector.tensor_reduce(mx[:], a_ps[:], AX.X, ALU.max,
                                                negate=True)
                        mx2 = asb.tile([128, 1], F32, tag="mx2", name="mx2")
                        nc.scalar.mul(mx2[:], mx[:], SCALE)
                        ex = asb.tile([128, S], F32, tag="ex", name="ex")
                        den = asb.tile([128, 1], F32, tag="den", name="den")
                        nc.scalar.activation(ex[:], a_ps[:], AF.Exp, bias=mx2[:],
                                             scale=SCALE, accum_out=den[:])
                        rec = asb.tile([128, 1], F32, tag="rec", name="rec")
                        nc.vector.reciprocal(rec[:], den[:])
                        p_sb = asb.tile([128, S], BF16, tag="p", name="p")
                        nc.vector.tensor_scalar_mul(p_sb[:], ex[:], rec[:])
                        pt_ps = tpsp.tile([128, 128], BF16, tag="tp", name="pt",
                                          padded_shape=[128, 128])
                        nc.tensor.transpose(pt_ps[:], p_sb[:], ident[:])
                        pT = asb.tile([128, 128], BF16, tag="pT", name="pT")
                        nc.vector.tensor_copy(pT[:], pt_ps[:])
                        pT_t[(tau, bg)] = pT
                    return emit

                def unit_ctx(tau, bg):
                    def emit():
                        t0, Tcb = BLOCKS[tau]
                        b0 = bg * BG
                        pT = pT_t.pop((tau, bg))
                        Cb = cblk_t[tau]
                        for j in range(BG):
                            b = b0 + j
                            for k in range(KD):
                                c_ps = spsp.tile([128, 512], F32, tag="sp",
                                                 name="cx")
                                nc.tensor.matmul(
                                    c_ps[:, 0:Tcb],
                                    memN[:, (b * 2 * KE + k) * 128:
                                         (b * 2 * KE + k + 1) * 128],
                                    pT[:, 32 * j:32 * j + Tcb],
                                    start=True, stop=True)
                                nc.scalar.copy(
                                    Cb[:, k, b * Tcb:(b + 1) * Tcb],
                                    c_ps[:, 0:Tcb])
                    return emit

                def unit_mid(tau, half, m):
                    def emit():
                        t0, Tcb = BLOCKS[tau]
                        Hb = hblk_t[tau]
                        Hf = Hb[:].rearrange("p (k bt) -> p k bt", k=KD)
                        Cb = cblk_t[tau]
                        Mb = mblk_t[tau]
                        ps = spsp.tile([128, 512], F32, tag="sp", name="md")
                        c0 = half * 512
                        for k in range(KD):
                            lt = wf1_s[:, (k * KD + m) * 128:
                                       (k * KD + m + 1) * 128]
                            nc.tensor.matmul(ps[:], lt, Hf[:, k, c0:c0 + 512],
                                             start=(k == 0), stop=False)
                        for k in range(KD):
                            kk = KD + k
                            lt = wf1_s[:, (kk * KD + m) * 128:
                                       (kk * KD + m + 1) * 128]
                            nc.tensor.matmul(ps[:], lt, Cb[:, k, c0:c0 + 512],
                                             start=False, stop=(k == KD - 1))
                        ab = mlsb.tile([128, 512], F32, tag="mab", name="mab")
                        nc.scalar.activation(ab[:], ps[:], AF.Abs,
                                             bias=b1a_s[:, m:m + 1])
                        idt = mlsb.tile([128, 512], F32, tag="mid", name="mid")
                        nc.scalar.activation(idt[:], ps[:], AF.Identity,
                                             scale=0.55, bias=b1h_s[:, m:m + 1])
                        nc.vector.scalar_tensor_tensor(
                            Mb[:, m, c0:c0 + 512], ab[:], 0.45, idt[:],
                            ALU.mult, ALU.add)
                    return emit

                def unit_vocab(tau, btm, hf):
                    def emit():
                        t0, Tcb = BLOCKS[tau]
                        nb = 128 // Tcb
                        Mb = mblk_t[tau]
                        wf2v = wf2_s[:].rearrange("p (k v) -> p k v", k=KD)
                        b0 = btm * nb
                        HV = VS // 2
                        lgrow = lgp.tile([128, HV], BF16, tag="lgr",
                                         name="lgr")
                        for cch in range(HV // 500):
                            w0 = hf * HV + cch * 500
                            lg = spsp.tile([128, 512], F32, tag="sp",
                                           name="lg")
                            for k in range(KD):
                                nc.tensor.matmul(
                                    lg[:, 0:500],
                                    Mb[:, k, btm * 128:(btm + 1) * 128],
                                    wf2v[:, k, w0:w0 + 500],
                                    start=(k == 0), stop=(k == KD - 1))
                            nc.vector.tensor_copy(
                                lgrow[:, cch * 500:(cch + 1) * 500],
                                lg[:, 0:500])
                        for j in range(nb):
                            nc.sync.dma_start(
                                out.ap()[b0 + j, t0:t0 + Tcb,
                                         hf * HV:(hf + 1) * HV],
                                lgrow[Tcb * j:Tcb * (j + 1), :])
                    return emit

                MHD = 3 * MD // 4

                def block_units(tau):
                    t0, Tcb = BLOCKS[tau]
                    u = []
                    for bg in range(B // BG):
                        u.append(unit_score(tau, bg))
                        u.append(unit_ctx(tau, bg))
                    for half in range(B * Tcb // 512):
                        for m in range(KD):
                            u.append(unit_mid(tau, half, m))
                    for btm in range(B * Tcb // 128):
                        for hf in range(2):
                            u.append(unit_vocab(tau, btm, hf))
                    return u

                def alloc_blk(tau):
                    t0, Tcb = BLOCKS[tau]
                    cblk_t[tau] = cbp.tile([128, KD, B * Tcb], BF16,
                                           tag="cb", name=f"cb{tau}")
                    if phases >= 6:
                        mblk_t[tau] = mbp.tile([128, KD, B * Tcb], BF16,
                                               tag="mb", name=f"mb{tau}")

                queue = [unit_memN(b) for b in range(B)] if phases >= 5 else []
                for tau, (t0, Tcb) in enumerate(BLOCKS):
                    hblk_t[tau] = hbp.tile([128, KD * B * Tcb], BF16,
                                           tag="hb", name=f"hb{tau}")
                    if tau >= 1 and phases >= 5:
                        alloc_blk(tau - 1)
                        queue.extend(block_units(tau - 1) if phases >= 6 else
                                     [unit_score(tau - 1, bg)
                                      for bg in range(B // BG)])
                    for trel in range(Tcb):
                        t = t0 + trel
                        hcur = h_d[t % 2]
                        hnxt = h_d[(t + 1) % 2]
                        emb = decT[:, 0, t * B:(t + 1) * B]
                        psa = dpsa.tile([128, MHD * 32], F32, tag="gda",
                                        name="gda")
                        psb = dpsb.tile([128, (MD - MHD) * 32], F32, tag="gdb",
                                        name="gdb")
                        for mi in range(MD):
                            ps = psa if mi < MHD else psb
                            oc = ps[:, (mi % MHD) * 32:(mi % MHD + 1) * 32]
                            nc.tensor.matmul(
                                oc, wih_d_s[:, mi * 128:(mi + 1) * 128],
                                emb, start=True, stop=False)
                            for k in range(KD):
                                nc.tensor.matmul(
                                    oc,
                                    whh_d_s[:, (mi * KD + k) * 128:
                                            (mi * KD + k + 1) * 128],
                                    hcur[:, k * B:(k + 1) * B],
                                    start=False,
                                    stop=(k == KD - 1 and not bias_mode))
                            if bias_mode:
                                nc.tensor.matmul(
                                    oc, brow_d_s[:, mi * 128:(mi + 1) * 128],
                                    ones_s[:, :], start=False, stop=True)
                        # bank A: [i f o] in tanh form (sigmoid via
                        # tanh(x/2); factor-2s folded into host weights:
                        # state is cc=2c, h2=2h); bank B: [g]
                        th = dsb.tile([128, 3 * GD], F32, tag="sgd", name="sgd")
                        nc.scalar.activation(th[:], psa[:], AF.Tanh, scale=0.25)
                        gt = dsb.tile([128, GD], F32, tag="gtd", name="gtd")
                        nc.scalar.activation(gt[:], psb[:, 0:GD], AF.Tanh,
                                             scale=0.5)
                        t1 = dsb.tile([128, GD], F32, tag="t1d", name="t1d")
                        nc.vector.scalar_tensor_tensor(
                            t1[:], th[:, GD:2 * GD], 1.0, c_d[:],
                            ALU.add, ALU.mult)
                        t2 = dsb.tile([128, GD], F32, tag="t2d", name="t2d")
                        nc.vector.scalar_tensor_tensor(
                            t2[:], th[:, 0:GD], 1.0, gt[:], ALU.add, ALU.mult)
                        nc.vector.scalar_tensor_tensor(
                            c_d[:], t1[:], 0.5, t2[:], ALU.mult, ALU.add)
                        tct = dsb.tile([128, GD], F32, tag="tcd", name="tcd")
                        nc.scalar.activation(tct[:], c_d[:], AF.Tanh, scale=0.5)
                        nc.vector.scalar_tensor_tensor(
                            hnxt[:], th[:, 2 * GD:3 * GD], 1.0, tct[:],
                            ALU.add, ALU.mult)
                        Hb4 = hblk_t[tau][:].rearrange("p (k b t) -> p k b t",
                                                       k=KD, b=B)
                        nc.vector.tensor_copy(
                            Hb4[:, :, :, trel],
                            hnxt[:].rearrange("p (k b) -> p k b", k=KD))
                        # paced interleave of previous block's post-work
                        rem_steps = Tcb - trel
                        npop = (len(queue) + rem_steps - 1) // rem_steps
                        for _ in range(npop):
                            queue.pop(0)()

                # tail: last block's post-work
                if phases >= 5:
                    tau = len(BLOCKS) - 1
                    alloc_blk(tau)
                    for u in (block_units(tau) if phases >= 6 else
                              [unit_score(tau, bg) for bg in range(B // BG)]):
                        u()
        except _Stop:
            pass
    nc.compile()
    return nc


def prep_inputs(i, S=128, T=128, V=32000, VS=4000, fp8=False):
    def as_np(x, dt=np.float32):
        return np.ascontiguousarray(np.asarray(x), dtype=dt)

    whh_np = f8e4 if fp8 else bf16
    tok = as_np(i["tok_emb"]).astype(bf16)

    def idx_prep(flat):
        a = flat.astype(np.int16).reshape(-1, 16).T
        return np.ascontiguousarray(np.tile(a, (8, 1)))

    inp = as_np(i["inp"], np.int64)
    x = as_np(i["x"], np.int64)
    idx_e = idx_prep(inp.T.reshape(-1))
    dmat = np.zeros((B, T), np.int64)
    dmat[:, 1:] = x[:, :T - 1]
    idx_d = idx_prep(dmat.T.reshape(-1))

    startT = as_np(i["start_emb"]).reshape(D_EMB, 1).astype(bf16)
    est = as_np(i["enc_style_emb"])
    diff_e = (est[1] - est[0]).reshape(1, -1).astype(bf16)
    e0T = np.ascontiguousarray(est[0].reshape(KD, 128).T)
    sty = 2.0 * as_np(i["style_emb"])  # decoder h-state kept as 2h
    diff_s = (sty[1] - sty[0]).reshape(1, -1).astype(bf16)
    s0T = np.ascontiguousarray(sty[0].reshape(KD, 128).T)
    lab_i = as_np(i["label_i"], np.float32).reshape(1, B).astype(bf16)
    lab_d = as_np(i["label"], np.float32).reshape(1, B).astype(bf16)

    def wihP(w, nm, perm):
        # w [4H, 128] -> [128, nm*128], tile mi = chunk perm[mi], lhsT layout
        a = w.reshape(nm, 128, 128)          # [m, out, in]
        a = a[perm]                          # permuted
        return np.ascontiguousarray(a.transpose(2, 0, 1).reshape(128, nm * 128)
                                    ).astype(bf16)

    def whhP(w, nk, nm, perm):
        # w [4H, H] -> [128, nm*nk*128], (m-perm, k)-major
        a = w.reshape(nm, 128, nk, 128)      # [m, out, k, in]
        a = a[perm]
        a = a.transpose(3, 0, 2, 1)          # [in, m, k, out]
        return np.ascontiguousarray(a.reshape(128, nm * nk * 128)).astype(whh_np)

    def whhT(w, nk, nm):
        # (k, m)-major, unpermuted (for wtr / wf1)
        a = w.reshape(nm, 128, nk, 128)
        a = a.transpose(3, 2, 0, 1)
        return np.ascontiguousarray(a.reshape(128, nk * nm * 128)).astype(bf16)

    wih_f = wihP(as_np(i["Wih_f"]), ME, PERM_E)
    wih_b = wihP(as_np(i["Wih_b"]), ME, PERM_E)
    wih_d = wihP(2.0 * as_np(i["Wih_d"]), MD, PERM_D)
    whh_f = whhP(as_np(i["Whh_f"]), KE, ME, PERM_E)
    whh_b = whhP(as_np(i["Whh_b"]), KE, ME, PERM_E)
    whh_d = whhP(as_np(i["Whh_d"]), KD, MD, PERM_D)
    wtr = whhT(2.0 * as_np(i["W_tr"]), KD, KD)
    wf1_np = as_np(i["W_f1"]).copy()
    wf1_np[:, :D_DEC] *= 0.5          # h-input half reads h2 = 2h
    wf1 = whhT(wf1_np, 8, KD)
    wf2_full = as_np(i["W_f2"])
    b1 = as_np(i["b_f1"])
    b1a = np.ascontiguousarray(b1.reshape(KD, 128).T)
    b1h = np.ascontiguousarray((0.55 * b1).reshape(KD, 128).T)

    bs_f = as_np(i["bih_f"]) + as_np(i["bhh_f"])
    bs_b = as_np(i["bih_b"]) + as_np(i["bhh_b"])
    bs_d = as_np(i["bih_d"]) + as_np(i["bhh_d"])
    bias_mode = bool(np.any(bs_f) or np.any(bs_b) or np.any(bs_d))

    common = dict(tokb=tok, idx_e=idx_e, idx_d=idx_d, startT=startT,
                  diff_e=diff_e, e0T=e0T, lab_i=lab_i,
                  diff_s=diff_s, s0T=s0T, lab_d=lab_d,
                  wih_f=wih_f, wih_b=wih_b, wih_d=wih_d,
                  whh_f=whh_f, whh_b=whh_b, whh_d=whh_d,
                  wtr=wtr, wf1=wf1, b1a=b1a, b1h=b1h)
    if bias_mode:
        def brow(v, nm, perm):
            a = v.reshape(nm, 128)[perm]
            return np.ascontiguousarray(a.reshape(1, nm * 128)).astype(bf16)
        common.update(brow_f=brow(bs_f, ME, PERM_E), brow_b=brow(bs_b, ME, PERM_E),
                      brow_d=brow(2.0 * bs_d, MD, PERM_D))

    in_maps = []
    for c in range(N_CORES):
        shard = wf2_full[c * VS:(c + 1) * VS]
        a = shard.reshape(VS, KD, 128)
        wf2 = np.ascontiguousarray(a.transpose(2, 1, 0).reshape(128, KD * VS)
                                   ).astype(bf16)
        in_maps.append(dict(common, wf2=wf2))
    return in_maps, bias_mode


_NC_CACHE = {}
_FP8 = True


def kernel(**inputs):
    in_maps, bias_mode = prep_inputs(inputs, fp8=_FP8)
    key = (bias_mode, _FP8)
    if key not in _NC_CACHE:
        _NC_CACHE[key] = build(fp8=_FP8, bias_mode=bias_mode)
    nc = _NC_CACHE[key]
    res = run_bass_kernel_spmd(nc, in_maps, core_ids=list(range(N_CORES)))
    return np.concatenate([r["out"].astype(np.float32) for r in res.results],
                          axis=2)
